# revision 1
# baseline (speedup 1.0000x reference)
"""Trainium2 Bass kernel for nn_CBFLayer (batch CBF-QP safety filter).

Contract: kernel(u_nom, obs) takes FULL inputs (numpy), returns FULL output.
Internally: pure data-parallel shard of the batch across 8 NeuronCores.

Math (per sample, exact KKT of the QP  min |u-u_nom|^2 + LAM*s^2
s.t. a@u <= b+s, |u|^2 <= 1, s >= 0, with a = -2*g, g = p_rel):
  u = (u_nom + 2*t*g) * rho,  rho = min(1/||u_nom + 2*t*g||, 1)
where t >= 0 is the CBF multiplier (t = mu1/2):
  - case1 (constraint slack at t=0):            t = 0
  - case2 (CBF active, ball inactive):          t = t2 (exact linear root)
  - case3 (both active): root of
      phi(t) = (p - t*A) - (b + t/LAM)*||u_nom - t*a||
    found with a pole-regularized geometric seed + 1 Newton + 1 chord step.
All transcendentals (sqrt / rsqrt / reciprocal / x^(2/3)) are computed as
Exp(k*Ln(x)) so the whole kernel needs exactly ONE ScalarE table set
(natural_log_exp_and_others: ln, exp, square, abs, relu, copy, identity);
_PinnedBacc forces that set so the compiler cannot thrash table loads.
"""

import numpy as np

B = 4194304
NCORES = 8
BC = B // NCORES            # 524288 samples per core
P = 128
NPER = BC // P              # 4096 samples per partition
KC = 512                    # compute-tile samples per partition
NT = NPER // KC             # tiles per core

LAM = 10000.0
TOL = 1e-6

_CACHE = {}


def _build():
    import bass_rust as _bass_rust
    import concourse.bacc as bacc
    import concourse.mybir as mybir
    from concourse.tile import TileContext
    from concourse.hw_specs import get_activation_tables

    F32 = mybir.dt.float32
    OP = mybir.AluOpType
    AF = mybir.ActivationFunctionType

    class _PinnedBacc(bacc.Bacc):
        """Bacc whose activation-table chooser only sees
        natural_log_exp_and_others (list order preserved so
        act_func_set_id indices stay aligned with act_info.json)."""

        def insert_act_table_loads(self):
            has_activation = any(
                isinstance(i, mybir.InstActivation)
                for b in self.main_func.blocks
                for i in b.instructions
            )
            if not has_activation:
                return
            tables = [
                (k, v if k == "natural_log_exp_and_others" else set())
                for k, v in get_activation_tables(self.m.arch).items()
            ]
            _bass_rust.insert_act_table_loads(self, tables)

    nc = _PinnedBacc("TRN2", target_bir_lowering=False, debug=False)
    pk_in = nc.dram_tensor("pk", [P, NPER * 6], F32, kind="ExternalInput").ap()
    out_d = nc.dram_tensor("out", [P, NPER * 2], F32, kind="ExternalOutput").ap()

    V = "V"  # DVE vector engine
    G = "G"  # Pool / gpsimd engine

    WST_BIAS = -(2.0 / 3.0) * float(np.log(2.0 * LAM))

    def register_const(value):
        t = nc.alloc_sbuf_tensor(f"const-f32-{value}", [P, 1], F32)
        nc.gpsimd.memset(t.ap(), value)
        nc.const_aps.aps[(F32, value)] = t.ap()

    register_const(WST_BIAS)
    nc.all_engine_barrier()

    with TileContext(nc) as tc:
        with (
            tc.tile_pool(name="io", bufs=2) as io,
            tc.tile_pool(name="wk", bufs=2) as wk,       # persist + newton scratch
            tc.tile_pool(name="ck", bufs=1) as ck,       # chain-local scratch
        ):
            def eng(e):
                return {"V": nc.vector, "G": nc.gpsimd}[e]

            def tt(e, out, a, b, op):
                eng(e).tensor_tensor(out[:], a[:], b[:], op)

            def ts(e, out, a, s1, op0, s2=None, op1=None):
                if op1 is None:
                    eng(e).tensor_scalar(out[:], a[:], s1, None, op0)
                else:
                    eng(e).tensor_scalar(out[:], a[:], s1, s2, op0, op1)

            def act(out, a, func, scale=1.0, bias=0.0):
                nc.scalar.activation(out[:], a[:], func, bias=bias, scale=scale)

            def mul(e, out, a, b):
                tt(e, out, a, b, OP.mult)

            def add(e, out, a, b):
                tt(e, out, a, b, OP.add)

            def sub(e, out, a, b):
                tt(e, out, a, b, OP.subtract)

            for i in range(NT):
                # ---------------- load (single packed DMA) ----------------
                pk_t = io.tile([P, 6 * KC], F32, tag="pk_t")
                o_t = io.tile([P, 2 * KC], F32, tag="o_t")
                nc.sync.dma_start(out=pk_t[:], in_=pk_in[:, i * 6 * KC:(i + 1) * 6 * KC])
                uxs = pk_t[:, 0:2 * KC:2]
                uys = pk_t[:, 1:2 * KC:2]
                gx = pk_t[:, 2 * KC:3 * KC]
                gy = pk_t[:, 3 * KC:4 * KC]
                vx = pk_t[:, 4 * KC:5 * KC]
                vy = pk_t[:, 5 * KC:6 * KC]
                oxs = o_t[:, 0:2 * KC:2]
                oys = o_t[:, 1:2 * KC:2]

                def T(name, tag=None):
                    return wk.tile([P, KC], F32, tag=tag or name, name=name)

                def C(name, tag=None):
                    return ck.tile([P, KC], F32, tag=tag or name, name=name)

                # ---------------- derived ----------------
                gx2 = C("gx2", "ckA"); act(gx2, gx, AF.Square)
                gy2 = C("gy2", "ckB"); act(gy2, gy, AF.Square)
                S = T("S"); add(V, S, gx2, gy2)
                m1 = C("m1", "ckA"); mul(V, m1, gx, uxs)
                m2 = C("m2", "ckB"); mul(V, m2, gy, uys)
                P2 = T("P2"); add(V, P2, m1, m2)
                r1 = C("r1", "ckC"); mul(G, r1, gx, vx)
                r2 = C("r2", "ckD"); mul(G, r2, gy, vy)
                pv = r1; add(G, pv, r1, r2)
                ux2 = C("ux2", "ckB"); act(ux2, uxs, AF.Square)
                uy2 = C("uy2", "ckA"); act(uy2, uys, AF.Square)
                N = T("N"); add(V, N, ux2, uy2)
                bh = C("bh", "ckD"); sub(V, bh, S, pv)
                b1 = T("b1"); act(b1, bh, AF.Copy, bias=-1.0)
                A4 = T("A4"); act(A4, S, AF.Copy, scale=4.0)
                p = T("p"); act(p, P2, AF.Copy, scale=-2.0)
                b2 = T("b2"); act(b2, b1, AF.Copy, scale=2.0)
                cm1 = C("cm1", "ckB"); mul(G, cm1, gy, uxs)
                cm2 = C("cm2", "ckA"); mul(G, cm2, gx, uys)
                cr = T("cr"); sub(G, cr, cm1, cm2)

                # ---------------- feas1 mask ----------------
                lnN = C("lnN", "ckE"); act(lnN, N, AF.Ln)
                sqN = C("sqN", "ckF"); act(sqN, lnN, AF.Exp, scale=0.5)
                mn = C("mn", "ckG"); ts(V, mn, sqN, 1.0, OP.min)
                lhs = C("lhs", "ckE"); mul(G, lhs, p, mn)
                b2t = C("b2t", "ckH"); act(b2t, b2, AF.Copy, bias=TOL)
                rhs = C("rhs", "ckG"); mul(G, rhs, b2t, sqN)
                dd = C("dd", "ckF"); sub(G, dd, lhs, rhs)
                nf1 = T("nf1"); ts(V, nf1, dd, 0.0, OP.is_gt)

                # ---------------- t_lin (case2) + ball check ----------------
                den = C("den", "ckI"); act(den, S, AF.Copy, scale=4.0 * LAM, bias=1.0)
                lnd = C("lnd", "ckJ"); act(lnd, den, AF.Ln)
                rden = C("rden", "ckI"); act(rden, lnd, AF.Exp, scale=-1.0)
                num = C("num", "ckJ"); add(G, num, P2, b1)
                t2a = C("t2a", "ckK"); mul(G, t2a, num, rden)
                t2 = T("t2"); act(t2, t2a, AF.Copy, scale=-2.0 * LAM)
                zq = C("zq", "ckI"); mul(G, zq, t2, A4)
                zqp = C("zqp", "ckJ"); sub(G, zqp, zq, p)
                zqpp = C("zqpp", "ckK"); sub(G, zqpp, zqp, p)
                zm = C("zm", "ckI"); mul(G, zm, t2, zqpp)
                n2 = C("n2", "ckJ"); add(G, n2, N, zm)
                mA = C("mA", "ckK"); ts(V, mA, t2, -TOL, OP.is_ge)
                mB = C("mB", "ckL"); ts(V, mB, n2, 1.0 + TOL, OP.is_le)
                ok2 = T("ok2"); mul(G, ok2, mA, mB)
                no2 = C("no2", "ckL"); act(no2, ok2, AF.Copy, scale=-1.0, bias=1.0)
                nm = T("nm"); mul(G, nm, nf1, no2)

                # ---------------- case3 geometric seed (pole-floored) ----------
                Scl = C("Scl", "ckM"); ts(V, Scl, S, 1e-30, OP.max)
                lnS = C("lnS", "ckN"); act(lnS, Scl, AF.Ln)
                rS = C("rS", "ckM"); act(rS, lnS, AF.Exp, scale=-0.5)
                rS2 = C("rS2", "ckO"); act(rS2, lnS, AF.Exp, scale=-1.0)
                sqS = C("sqS", "ckP"); act(sqS, lnS, AF.Exp, scale=0.5)
                beta = C("beta", "ckQ"); mul(G, beta, b1, rS)
                ta1 = C("ta1", "ckN"); add(G, ta1, b1, sqS)
                talt = C("talt", "ckR"); act(talt, ta1, AF.Relu, scale=-2.0 * LAM)
                bsq = C("bsq", "ckM"); act(bsq, beta, AF.Square)
                w2 = C("w2", "ckS"); act(w2, bsq, AF.Copy, scale=-1.0, bias=1.0)
                acr = C("acr", "ckT"); act(acr, cr, AF.Abs)
                lcr = C("lcr", "ckU"); act(lcr, acr, AF.Ln)
                wst = C("wst", "ckP"); act(wst, lcr, AF.Exp, scale=2.0 / 3.0, bias=WST_BIAS)
                ws2 = C("ws2", "ckM"); mul(V, ws2, wst, rS2)
                w2c = C("w2c", "ckP"); tt(V, w2c, w2, ws2, OP.max)
                ts(V, w2c, w2c, 1e-12, OP.max)
                lnw = C("lnw", "ckS"); act(lnw, w2c, AF.Ln)
                rw = C("rw", "ckM"); act(rw, lnw, AF.Exp, scale=-0.5)
                km = C("km", "ckS"); mul(V, km, acr, rw)
                km2 = C("km2", "ckP"); mul(V, km2, km, beta)
                sm = C("sm", "ckM"); add(V, sm, P2, km2)
                pS = C("pS", "ckQ"); mul(G, pS, p, rS2)
                tm1 = C("tm1", "ckS"); mul(V, tm1, sm, rS2)
                tmain = C("tmain", "ckP"); act(tmain, tm1, AF.Copy, scale=-0.5)
                tc1 = C("tc1", "ckM"); act(tc1, b2, AF.Copy, scale=-LAM)
                tc2 = C("tc2", "ckS"); act(tc2, pS, AF.Copy, scale=0.25)
                tcm = C("tcm", "ckU"); tt(V, tcm, tc1, tc2, OP.max)
                tcr = C("tcr", "ckQ"); act(tcr, tcm, AF.Relu)
                t = T("t"); tt(V, t, tmain, talt, OP.max)
                tt(V, t, t, tcr, OP.min)
                nc.vector.copy_predicated(t[:], ok2[:].bitcast(mybir.dt.uint32), t2[:])
                mul(V, t, t, nf1)

                # ---------------- Newton (full) ----------------
                q = T("w1"); mul(V, q, t, A4)
                qp = T("w2"); sub(V, qp, q, p)
                qpp = T("w3"); sub(V, qpp, qp, p)
                mm = T("w4"); mul(V, mm, t, qpp)
                nn = T("w5"); add(V, nn, N, mm)
                ts(V, nn, nn, 1e-12, OP.max)
                lnn = T("w6"); act(lnn, nn, AF.Ln)
                rn = T("w7"); act(rn, lnn, AF.Exp, scale=-0.5)
                nrm = T("w8"); mul(V, nrm, nn, rn)
                bt = T("bt", "w4"); act(bt, t, AF.Copy, scale=1.0 / LAM)
                bb = T("bb", "w3"); add(V, bb, b2, bt)
                fb = T("fb", "w6"); mul(V, fb, bb, nrm)
                phin = T("phin", "w1"); add(V, phin, qp, fb)
                d1 = T("d1", "w9"); act(d1, nrm, AF.Copy, scale=1.0 / LAM)
                e1 = T("e1", "w10"); mul(G, e1, bb, qp)
                mul(G, e1, e1, rn)
                add(G, d1, A4, d1)
                add(G, d1, d1, e1)
                ts(V, d1, d1, 1e-8, OP.max)
                ls2 = T("ls2", "w10"); act(ls2, d1, AF.Ln)
                rdf = T("rdf"); act(rdf, ls2, AF.Exp, scale=-1.0)
                mul(G, rdf, rdf, nm)
                dl = T("dl", "w8"); mul(V, dl, phin, rdf)
                sub(V, t, t, dl)
                act(t, t, AF.Relu)

                # ---------------- chord ----------------
                q2 = T("q2", "w1"); mul(V, q2, t, A4)
                qpc = T("qpc", "w2"); sub(V, qpc, q2, p)
                qppc = T("qppc", "w3"); sub(V, qppc, qpc, p)
                mmc = T("mmc", "w4"); mul(V, mmc, t, qppc)
                nnc = T("nnc", "w5"); add(V, nnc, N, mmc)
                ts(V, nnc, nnc, 1e-12, OP.max)
                lnn2 = T("lnn2", "w6"); act(lnn2, nnc, AF.Ln)
                rn2 = T("rn2", "w7"); act(rn2, lnn2, AF.Exp, scale=-0.5)
                nrm2 = T("nrm2", "w8"); mul(V, nrm2, nnc, rn2)
                btc = T("btc", "w4"); act(btc, t, AF.Copy, scale=1.0 / LAM)
                bbc = T("bbc", "w3"); add(V, bbc, b2, btc)
                fbc = T("fbc", "w6"); mul(V, fbc, bbc, nrm2)
                phin2 = T("phin2", "w1"); add(V, phin2, qpc, fbc)
                dl2 = T("dl2", "w8"); mul(V, dl2, phin2, rdf)
                sub(V, t, t, dl2)
                act(t, t, AF.Relu)

                # ---------------- final rho ----------------
                qf = T("qf", "w1"); mul(V, qf, t, A4)
                qpf = T("qpf", "w2"); sub(V, qpf, qf, p)
                qppf = T("qppf", "w3"); sub(V, qppf, qpf, p)
                mmf = T("mmf", "w4"); mul(V, mmf, t, qppf)
                nnf = T("nnf", "w5"); add(V, nnf, N, mmf)
                ts(V, nnf, nnf, 1e-12, OP.max)
                lnnf = T("lnnf", "w6"); act(lnnf, nnf, AF.Ln)
                rnf = T("rnf", "w7"); act(rnf, lnnf, AF.Exp, scale=-0.5)
                rho = T("rho"); ts(V, rho, rnf, 1.0, OP.min)

                # ---------------- assembly ----------------
                tx2 = T("tx2", "w2"); act(tx2, t, AF.Copy, scale=2.0)
                ax = T("ax", "w1"); mul(V, ax, tx2, gx)
                sx = T("sx", "w3"); add(V, sx, uxs, ax)
                nc.vector.tensor_tensor(oxs, sx[:], rho[:], OP.mult)
                ay = T("ay", "w4"); mul(G, ay, tx2, gy)
                sy = T("sy", "w6"); add(G, sy, uys, ay)
                nc.gpsimd.tensor_tensor(oys, sy[:], rho[:], OP.mult)

                nc.sync.dma_start(out=out_d[:, i * 2 * KC:(i + 1) * 2 * KC], in_=o_t[:])

    nc.compile()
    return nc


def _get_nc():
    if "nc" not in _CACHE:
        _CACHE["nc"] = _build()
    return _CACHE["nc"]


def _run(u_nom: np.ndarray, obs: np.ndarray, trace: bool = False):
    from concourse.bass_utils import run_bass_kernel_spmd

    u_nom = np.asarray(u_nom, dtype=np.float32)
    obs = np.asarray(obs, dtype=np.float32)

    nc = _get_nc()
    in_maps = []
    for c in range(NCORES):
        s = slice(c * BC, (c + 1) * BC)
        uc = u_nom[s].reshape(P, NT, 2 * KC)
        oc = obs[s].reshape(P, NT, KC, 6)
        pk = np.concatenate(
            [uc,
             np.ascontiguousarray(oc[:, :, :, 2]),
             np.ascontiguousarray(oc[:, :, :, 3]),
             np.ascontiguousarray(oc[:, :, :, 4]),
             np.ascontiguousarray(oc[:, :, :, 5])],
            axis=2).reshape(P, NPER * 6)
        in_maps.append({"pk": pk})
    res = run_bass_kernel_spmd(nc, in_maps, core_ids=list(range(NCORES)),
                               trace=trace)
    out = np.empty((B, 2), dtype=np.float32)
    for c in range(NCORES):
        out[c * BC:(c + 1) * BC] = res.results[c]["out"].reshape(BC, 2)
    return out, res


def kernel(u_nom: np.ndarray, obs: np.ndarray) -> np.ndarray:
    return _run(u_nom, obs)[0]


if __name__ == "__main__":
    rng = np.random.default_rng(0)
    u = rng.standard_normal((B, 2), dtype=np.float32)
    o = rng.standard_normal((B, 6), dtype=np.float32)
    r = kernel(u, o)
    print(r.shape, r.dtype, r[:4])



# revision 6
# speedup vs baseline: 1.5562x; 1.5562x over previous
"""Trainium2 Bass kernel for nn_CBFLayer (batch CBF-QP safety filter).

Contract: kernel(u_nom, obs) takes FULL inputs (numpy), returns FULL output.
Internally: pure data-parallel shard of the batch across 8 NeuronCores.

Math (per sample, exact KKT of the QP  min |u-u_nom|^2 + LAM*s^2
s.t. a@u <= b+s, |u|^2 <= 1, s >= 0, with a = -2*g, g = p_rel):
  u = (u_nom + 2*t*g) * rho,  rho = min(1/||u_nom + 2*t*g||, 1)
with multiplier t chosen per KKT case:
  - case1 (feasible after ball projection):  t = 0
  - case2 (CBF active, ball inactive):       t = t2 (exact linear root)
  - case3 (both active): closed-form root of the s=0 circle equation
      z = c*|w|/sqrt(1-c^2), pole-floored by (|C|/(2*LAM))/S, plus the
      deep-infeasible branch t = LAM*relu(-(b/2+sqrt(S)))*2.
Seed-only (no Newton) is accurate to ~7e-4 rel which is far inside the
2e-2 gate.  All transcendentals are Sqrt activations (single table set
sqrt_and_others) + reciprocal_approx_fast custom-DVE ops; the affine+mul
chains are fused with scalar_tensor_tensor.
"""

import numpy as np

B = 4194304
NCORES = 8
BC = B // NCORES            # 524288 samples per core
P = 128
NPER = BC // P              # 4096 samples per partition
KC = 512                    # compute-tile samples per partition
NT = NPER // KC             # tiles per core

LAM = 10000.0
TOL = 1e-6

_CACHE = {}


def _build():
    import bass_rust as _bass_rust
    import concourse.bacc as bacc
    import concourse.mybir as mybir
    from concourse.tile import TileContext
    from concourse.hw_specs import get_activation_tables

    F32 = mybir.dt.float32
    OP = mybir.AluOpType
    AF = mybir.ActivationFunctionType

    class _PinnedBacc(bacc.Bacc):
        """Bacc whose activation-table chooser only sees sqrt_and_others
        (list order preserved so act_func_set_id indices stay aligned
        with act_info.json)."""

        def insert_act_table_loads(self):
            has_activation = any(
                isinstance(i, mybir.InstActivation)
                for b in self.main_func.blocks
                for i in b.instructions
            )
            if not has_activation:
                return
            tables = [
                (k, v if k == "sqrt_and_others" else set())
                for k, v in get_activation_tables(self.m.arch).items()
            ]
            _bass_rust.insert_act_table_loads(self, tables)

    nc = _PinnedBacc("TRN2", target_bir_lowering=False, debug=False)
    pk_in = nc.dram_tensor("pk", [P, NPER * 6], F32, kind="ExternalInput").ap()
    out_d = nc.dram_tensor("out", [P, NPER * 2], F32, kind="ExternalOutput").ap()

    def register_const(value):
        t = nc.alloc_sbuf_tensor(f"const-f32-{value}", [P, 1], F32)
        nc.gpsimd.memset(t.ap(), value)
        nc.const_aps.aps[(F32, value)] = t.ap()

    register_const(0.0)
    nc.all_engine_barrier()

    with TileContext(nc) as tc:
        with (
            tc.tile_pool(name="io", bufs=2) as io,
            tc.tile_pool(name="wk", bufs=2) as wk,       # cross-stage values
            tc.tile_pool(name="ck", bufs=1) as ck,       # short-lived scratch
        ):
            def eng(e):
                return {"V": nc.vector, "G": nc.gpsimd}[e]

            def tt(e, out, a, b, op):
                eng(e).tensor_tensor(out[:], a[:], b[:], op)

            def ts(e, out, a, s1, op0, s2=None, op1=None):
                if op1 is None:
                    eng(e).tensor_scalar(out[:], a[:], s1, None, op0)
                else:
                    eng(e).tensor_scalar(out[:], a[:], s1, s2, op0, op1)

            def stt(e, out, in0, s, in1, op0, op1):
                eng(e).scalar_tensor_tensor(out[:], in0[:], s, in1[:], op0, op1)

            def act(out, a, func, scale=1.0, bias=0.0):
                nc.scalar.activation(out[:], a[:], func, bias=bias, scale=scale)

            def rcp(out, in_):
                nc.vector.reciprocal_approx_fast(out=out[:], in_=in_[:])

            for i in range(NT):
                # ---------------- load (single packed DMA) ----------------
                # per-tile block layout: [gx gy | ux uy | vx vy], KC each
                pk_t = io.tile([P, 6 * KC], F32, tag="pk_t")
                o_t = io.tile([P, 2 * KC], F32, tag="o_t")
                nc.sync.dma_start(out=pk_t[:], in_=pk_in[:, i * 6 * KC:(i + 1) * 6 * KC])
                gb = pk_t[:, 0:2 * KC]
                ub = pk_t[:, 2 * KC:4 * KC]
                vb = pk_t[:, 4 * KC:6 * KC]
                gx = pk_t[:, 0:KC]
                gy = pk_t[:, KC:2 * KC]
                ux = pk_t[:, 2 * KC:3 * KC]
                uy = pk_t[:, 3 * KC:4 * KC]
                oxs = o_t[:, 0:KC]
                oys = o_t[:, KC:2 * KC]

                def W(name):
                    return wk.tile([P, KC], F32, tag=name, name=name)

                def C(name):
                    return ck.tile([P, KC], F32, tag=name, name=name)

                def C2T(name):
                    return ck.tile([P, 2 * KC], F32, tag=name, name=name)

                # ---------------- stage A: products & reductions ----------
                g2b = C2T("g2b"); act(g2b, gb, AF.Square)
                u2b = C2T("u2b"); act(u2b, ub, AF.Square)
                gub = C2T("gub"); tt("V", gub, gb, ub, OP.mult)
                gvb = C2T("gvb"); tt("G", gvb, gb, vb, OP.mult)
                S = W("S"); stt("V", S, g2b[:, 0:KC], 1e-30, g2b[:, KC:2 * KC], OP.add, OP.add)
                N = W("N"); tt("G", N, u2b[:, 0:KC], u2b[:, KC:2 * KC], OP.add)
                P_ = W("P"); tt("V", P_, gub[:, 0:KC], gub[:, KC:2 * KC], OP.add)
                Vd = C("Vd"); tt("G", Vd, gvb[:, 0:KC], gvb[:, KC:2 * KC], OP.add)
                b1 = W("b1"); stt("V", b1, S, -1.0, Vd, OP.add, OP.subtract)
                SN = C("SN"); tt("V", SN, S, N, OP.mult)
                Psq = C("Psq"); act(Psq, P_, AF.Square)
                C2m = C("C2m"); tt("G", C2m, SN, Psq, OP.subtract)
                C2r = C("C2r"); act(C2r, C2m, AF.Relu)
                acr = W("acr"); act(acr, C2r, AF.Sqrt)       # |C| = sqrt(S*N - P^2)

                # ---------------- scalars ----------------
                rS = W("rS"); rcp(rS, S)                     # 1/S
                sqS = W("sqS"); act(sqS, S, AF.Sqrt)
                isq = W("isq"); tt("V", isq, rS, sqS, OP.mult)   # 1/sqrt(S)
                iN = C("iN"); rcp(iN, N)
                rnN = C("rnN"); act(rnN, iN, AF.Sqrt)        # 1/sqrt(N)
                mn = C("mn"); ts("V", mn, rnN, 1.0, OP.min)

                # ---------------- feas1 mask ----------------
                pm = C("pm"); tt("G", pm, P_, mn, OP.mult)
                fd = C("fd"); tt("G", fd, b1, pm, OP.add)       # (b + 2 P mn)/2
                nf1 = W("nf1"); ts("V", nf1, fd, -0.5 * TOL, OP.is_lt)  # 1 => infeasible

                # ---------------- case 2 ----------------
                den = C("den"); ts("V", den, S, 4.0, OP.mult, 1e-4, OP.add)
                rden = C("rden"); rcp(rden, den)
                num = C("num"); tt("G", num, P_, b1, OP.add)
                t2 = W("t2"); stt("V", t2, num, -2.0, rden, OP.mult, OP.mult)
                w_ = C("w"); tt("V", w_, t2, S, OP.mult)
                w2 = C("w2"); tt("G", w2, P_, w_, OP.add)
                x2 = C("x2"); tt("V", x2, t2, w2, OP.mult)
                n2 = C("n2"); stt("V", n2, x2, 4.0, N, OP.mult, OP.add)
                mA = C("mA"); ts("V", mA, t2, -TOL, OP.is_ge)
                mB = C("mB"); ts("G", mB, n2, 1.0 + TOL, OP.is_le)
                ok2 = W("ok2"); tt("G", ok2, mA, mB, OP.mult)

                # ---------------- case-3 closed-form seed ----------------
                beta = W("beta"); tt("V", beta, b1, isq, OP.mult)    # b/(2 sqrt S)
                bsq = C("bsq"); act(bsq, beta, AF.Square)
                w2m = C("w2m"); ts("G", w2m, bsq, -1.0, OP.mult, 1.0, OP.add)  # 1-beta^2
                ws2 = C("ws2"); stt("V", ws2, acr, 1.0 / (2.0 * LAM), rS, OP.mult, OP.mult)
                w2c = C("w2c"); stt("V", w2c, w2m, 1e-12, ws2, OP.max, OP.max)
                iw = C("iw"); rcp(iw, w2c)
                rw = C("rw"); act(rw, iw, AF.Sqrt)           # rsqrt(w2c)
                km = C("km"); tt("V", km, acr, rw, OP.mult)
                km2 = C("km2"); tt("G", km2, km, beta, OP.mult)
                sm = C("sm"); tt("V", sm, P_, km2, OP.add)
                tmain = C("tmain"); stt("V", tmain, sm, -0.5, rS, OP.mult, OP.mult)
                ta1 = C("ta1"); tt("G", ta1, b1, sqS, OP.add)
                talt = C("talt"); act(talt, ta1, AF.Relu, scale=-2.0 * LAM)
                t = W("t"); tt("V", t, tmain, talt, OP.max)
                nc.vector.copy_predicated(t[:], ok2[:].bitcast(mybir.dt.uint32), t2[:])
                tt("G", t, t, nf1, OP.mult)

                # ---------------- final assembly ----------------
                tg = C("tg"); tt("G", tg, t, t, OP.add)         # 2t
                ax = C("ax"); tt("V", ax, tg, gx, OP.mult)
                sx = W("sx"); tt("G", sx, ux, ax, OP.add)
                ay = C("ay"); tt("G", ay, tg, gy, OP.mult)
                sy = W("sy"); tt("V", sy, uy, ay, OP.add)
                sx2 = C("sx2"); act(sx2, sx, AF.Square)
                sy2 = C("sy2"); act(sy2, sy, AF.Square)
                nnf = C("nnf"); stt("V", nnf, sx2, 1e-30, sy2, OP.add, OP.add)
                inf = C("inf"); rcp(inf, nnf)
                rho0 = C("rho0"); act(rho0, inf, AF.Sqrt)
                rho = W("rho"); ts("V", rho, rho0, 1.0, OP.min)
                nc.vector.tensor_tensor(oxs, sx[:], rho[:], OP.mult)
                nc.gpsimd.tensor_tensor(oys, sy[:], rho[:], OP.mult)

                nc.sync.dma_start(out=out_d[:, i * 2 * KC:(i + 1) * 2 * KC], in_=o_t[:])

    nc.compile()
    return nc


def _get_nc():
    if "nc" not in _CACHE:
        _CACHE["nc"] = _build()
    return _CACHE["nc"]


def _run(u_nom: np.ndarray, obs: np.ndarray, trace: bool = False):
    from concourse.bass_utils import run_bass_kernel_spmd

    u_nom = np.asarray(u_nom, dtype=np.float32)
    obs = np.asarray(obs, dtype=np.float32)

    nc = _get_nc()
    in_maps = []
    for c in range(NCORES):
        s = slice(c * BC, (c + 1) * BC)
        uc = u_nom[s].reshape(P, NT, KC, 2)
        oc = obs[s].reshape(P, NT, KC, 6)
        # per-tile block layout: [gx gy | ux uy | vx vy]
        pk = np.stack(
            [oc[:, :, :, 2], oc[:, :, :, 3],
             uc[:, :, :, 0], uc[:, :, :, 1],
             oc[:, :, :, 4], oc[:, :, :, 5]],
            axis=2).reshape(P, NPER * 6)
        in_maps.append({"pk": np.ascontiguousarray(pk)})
    res = run_bass_kernel_spmd(nc, in_maps, core_ids=list(range(NCORES)),
                               trace=trace)
    out = np.empty((B, 2), dtype=np.float32)
    for c in range(NCORES):
        r = res.results[c]["out"].reshape(P, NT, 2, KC)
        out[c * BC:(c + 1) * BC] = np.transpose(r, (0, 1, 3, 2)).reshape(BC, 2)
    return out, res


def kernel(u_nom: np.ndarray, obs: np.ndarray) -> np.ndarray:
    return _run(u_nom, obs)[0]


if __name__ == "__main__":
    rng = np.random.default_rng(0)
    u = rng.standard_normal((B, 2), dtype=np.float32)
    o = rng.standard_normal((B, 6), dtype=np.float32)
    r = kernel(u, o)
    print(r.shape, r.dtype, r[:4])


# revision 11
# speedup vs baseline: 1.9115x; 1.2283x over previous
"""Trainium2 Bass kernel for nn_CBFLayer (batch CBF-QP safety filter).

Contract: kernel(u_nom, obs) takes FULL inputs (numpy), returns FULL output.
Internally: pure data-parallel shard of the batch across 8 NeuronCores.

Math (per sample, exact KKT of the QP  min |u-u_nom|^2 + LAM*s^2
s.t. a@u <= b+s, |u|^2 <= 1, s >= 0, with a = -2*g, g = p_rel):
  u = (u_nom + 2*t*g) * rho,  rho = min(1/||u_nom + 2*t*g||, 1)
with multiplier t chosen per KKT case:
  - case1 (feasible after ball projection):  t = 0
  - case2 (CBF active, ball inactive):       t = t2 (exact linear root)
  - case3 (both active): closed-form root of the s=0 circle equation
      z = c*|w|/sqrt(1-c^2), pole-floored by (|C|/(2*LAM))/S, plus the
      deep-infeasible branch t = LAM*relu(-(b/2+sqrt(S)))*2.
Seed-only (no Newton) is accurate to ~7e-4 rel which is far inside the
2e-2 gate.  All transcendentals are Sqrt activations (single table set
sqrt_and_others) + reciprocal_approx_fast custom-DVE ops; the affine+mul
chains are fused with scalar_tensor_tensor.
"""

import numpy as np

B = 4194304
NCORES = 8
BC = B // NCORES            # 524288 samples per core
P = 128
NPER = BC // P              # 4096 samples per partition
KC = 512                    # compute-tile samples per partition
NT = NPER // KC             # tiles per core

LAM = 10000.0
TOL = 1e-6

_CACHE = {}


def _build():
    import bass_rust as _bass_rust
    import concourse.bacc as bacc
    import concourse.mybir as mybir
    from concourse.tile import TileContext
    from concourse.hw_specs import get_activation_tables

    F32 = mybir.dt.float32
    OP = mybir.AluOpType
    AF = mybir.ActivationFunctionType

    class _PinnedBacc(bacc.Bacc):
        """Bacc whose activation-table chooser only sees sqrt_and_others
        (list order preserved so act_func_set_id indices stay aligned
        with act_info.json)."""

        def insert_act_table_loads(self):
            has_activation = any(
                isinstance(i, mybir.InstActivation)
                for b in self.main_func.blocks
                for i in b.instructions
            )
            if not has_activation:
                return
            tables = [
                (k, v if k == "sqrt_and_others" else set())
                for k, v in get_activation_tables(self.m.arch).items()
            ]
            _bass_rust.insert_act_table_loads(self, tables)

    nc = _PinnedBacc("TRN2", target_bir_lowering=False, debug=False)
    pk_in = nc.dram_tensor("pk", [P, NPER * 6], F32, kind="ExternalInput").ap()
    out_d = nc.dram_tensor("out", [P, NPER * 2], F32, kind="ExternalOutput").ap()

    def register_const(value):
        t = nc.alloc_sbuf_tensor(f"const-f32-{value}", [P, 1], F32)
        nc.gpsimd.memset(t.ap(), value)
        nc.const_aps.aps[(F32, value)] = t.ap()

    register_const(0.0)
    nc.all_engine_barrier()

    with TileContext(nc) as tc:
        with (
            tc.tile_pool(name="io", bufs=2) as io,
            tc.tile_pool(name="wk", bufs=2) as wk,       # cross-stage values
            tc.tile_pool(name="ck", bufs=1) as ck,       # short-lived scratch
        ):
            def eng(e):
                return {"V": nc.vector, "G": nc.gpsimd}[e]

            def tt(e, out, a, b, op):
                eng(e).tensor_tensor(out[:], a[:], b[:], op)

            def ts(e, out, a, s1, op0, s2=None, op1=None):
                if op1 is None:
                    eng(e).tensor_scalar(out[:], a[:], s1, None, op0)
                else:
                    eng(e).tensor_scalar(out[:], a[:], s1, s2, op0, op1)

            def stt(e, out, in0, s, in1, op0, op1):
                eng(e).scalar_tensor_tensor(out[:], in0[:], s, in1[:], op0, op1)

            def act(out, a, func, scale=1.0, bias=0.0):
                nc.scalar.activation(out[:], a[:], func, bias=bias, scale=scale)

            def rcp(out, in_):
                nc.vector.reciprocal_approx_fast(out=out[:], in_=in_[:])

            for i in range(NT):
                # ---------------- load (single packed DMA) ----------------
                # per-tile block layout: [gx gy | ux uy | vx vy], KC each
                pk_t = io.tile([P, 6 * KC], F32, tag="pk_t")
                o_t = io.tile([P, 2 * KC], F32, tag="o_t")
                nc.sync.dma_start(out=pk_t[:], in_=pk_in[:, i * 6 * KC:(i + 1) * 6 * KC])
                gb = pk_t[:, 0:2 * KC]
                ub = pk_t[:, 2 * KC:4 * KC]
                vb = pk_t[:, 4 * KC:6 * KC]
                gx = pk_t[:, 0:KC]
                gy = pk_t[:, KC:2 * KC]
                ux = pk_t[:, 2 * KC:3 * KC]
                uy = pk_t[:, 3 * KC:4 * KC]
                oxs = o_t[:, 0:KC]
                oys = o_t[:, KC:2 * KC]

                def W(name):
                    return wk.tile([P, KC], F32, tag=name, name=name)

                def C(name):
                    return ck.tile([P, KC], F32, tag=name, name=name)

                def C2T(name):
                    return ck.tile([P, 2 * KC], F32, tag=name, name=name)

                # ---------------- stage A: products & reductions ----------
                g2b = C2T("g2b"); act(g2b, gb, AF.Square)
                u2b = C2T("u2b"); act(u2b, ub, AF.Square)
                gub = C2T("gub"); tt("V", gub, gb, ub, OP.mult)
                gvb = C2T("gvb"); tt("G", gvb, gb, vb, OP.mult)
                S = W("S"); stt("V", S, g2b[:, 0:KC], 1e-30, g2b[:, KC:2 * KC], OP.add, OP.add)
                N = W("N"); tt("G", N, u2b[:, 0:KC], u2b[:, KC:2 * KC], OP.add)
                P_ = W("P"); tt("V", P_, gub[:, 0:KC], gub[:, KC:2 * KC], OP.add)
                Vd = C("Vd"); tt("G", Vd, gvb[:, 0:KC], gvb[:, KC:2 * KC], OP.add)
                b1 = W("b1"); stt("V", b1, S, -1.0, Vd, OP.add, OP.subtract)
                SN = C("SN"); tt("V", SN, S, N, OP.mult)
                Psq = C("Psq"); act(Psq, P_, AF.Square)
                C2m = C("C2m"); tt("G", C2m, SN, Psq, OP.subtract)
                C2r = C("C2r"); act(C2r, C2m, AF.Relu)
                acr = W("acr"); act(acr, C2r, AF.Sqrt)       # |C| = sqrt(S*N - P^2)

                # ---------------- scalars ----------------
                rS = W("rS"); rcp(rS, S)                     # 1/S
                sqS = W("sqS"); act(sqS, S, AF.Sqrt)
                isq = W("isq"); act(isq, rS, AF.Sqrt)        # 1/sqrt(S)
                iN = C("iN"); rcp(iN, N)
                rnN = C("rnN"); act(rnN, iN, AF.Sqrt)        # 1/sqrt(N)
                mn = C("mn"); ts("V", mn, rnN, 1.0, OP.min)

                # ---------------- feas1 mask ----------------
                pm = C("pm"); tt("G", pm, P_, mn, OP.mult)
                fd = C("fd"); tt("G", fd, b1, pm, OP.add)       # (b + 2 P mn)/2
                nf1 = W("nf1"); ts("V", nf1, fd, -0.5 * TOL, OP.is_lt)  # 1 => infeasible

                # ---------------- case 2 ----------------
                den = C("den"); act(den, S, AF.Copy, scale=4.0, bias=1e-4)
                rden = C("rden"); rcp(rden, den)
                num = C("num"); tt("G", num, P_, b1, OP.add)
                t2 = W("t2"); stt("V", t2, num, -2.0, rden, OP.mult, OP.mult)
                w_ = C("w"); tt("V", w_, t2, S, OP.mult)
                w2 = C("w2"); tt("G", w2, P_, w_, OP.add)
                x2 = C("x2"); tt("V", x2, t2, w2, OP.mult)
                n2 = C("n2"); stt("V", n2, x2, 4.0, N, OP.mult, OP.add)
                mA = C("mA"); ts("V", mA, t2, -TOL, OP.is_ge)
                mB = C("mB"); ts("V", mB, n2, 1.0 + TOL, OP.is_le)
                ok2 = W("ok2"); tt("G", ok2, mA, mB, OP.mult)

                # ---------------- case-3 closed-form seed ----------------
                beta = W("beta"); tt("V", beta, b1, isq, OP.mult)    # b/(2 sqrt S)
                bsq = C("bsq"); act(bsq, beta, AF.Square)
                w2m = C("w2m"); act(w2m, bsq, AF.Copy, scale=-1.0, bias=1.0)  # 1-beta^2
                ws2 = C("ws2"); stt("V", ws2, acr, 1.0 / (2.0 * LAM), rS, OP.mult, OP.mult)
                w2c = C("w2c"); stt("V", w2c, w2m, 1e-12, ws2, OP.max, OP.max)
                iw = C("iw"); rcp(iw, w2c)
                rw = C("rw"); act(rw, iw, AF.Sqrt)           # rsqrt(w2c)
                km = C("km"); tt("V", km, acr, rw, OP.mult)
                km2 = C("km2"); tt("G", km2, km, beta, OP.mult)
                sm = C("sm"); tt("V", sm, P_, km2, OP.add)
                tmain = C("tmain"); stt("V", tmain, sm, -0.5, rS, OP.mult, OP.mult)
                ta1 = C("ta1"); tt("G", ta1, b1, sqS, OP.add)
                talt = C("talt"); act(talt, ta1, AF.Relu, scale=-2.0 * LAM)
                t = W("t"); tt("V", t, tmain, talt, OP.max)
                nc.vector.copy_predicated(t[:], ok2[:].bitcast(mybir.dt.uint32), t2[:])
                tt("G", t, t, nf1, OP.mult)

                # ---------------- final assembly ----------------
                tg = C("tg"); act(tg, t, AF.Copy, scale=2.0)    # 2t
                ax = C("ax"); tt("V", ax, tg, gx, OP.mult)
                sx = W("sx"); tt("G", sx, ux, ax, OP.add)
                ay = C("ay"); tt("G", ay, tg, gy, OP.mult)
                sy = W("sy"); tt("V", sy, uy, ay, OP.add)
                sx2 = C("sx2"); act(sx2, sx, AF.Square)
                sy2 = C("sy2"); act(sy2, sy, AF.Square)
                nnf = C("nnf"); stt("V", nnf, sx2, 1e-30, sy2, OP.add, OP.add)
                inf = C("inf"); rcp(inf, nnf)
                rho0 = C("rho0"); act(rho0, inf, AF.Sqrt)
                rho = W("rho"); ts("V", rho, rho0, 1.0, OP.min)
                nc.vector.tensor_tensor(oxs, sx[:], rho[:], OP.mult)
                nc.gpsimd.tensor_tensor(oys, sy[:], rho[:], OP.mult)

                nc.sync.dma_start(out=out_d[:, i * 2 * KC:(i + 1) * 2 * KC], in_=o_t[:])

    nc.compile()
    return nc


def _get_nc():
    if "nc" not in _CACHE:
        _CACHE["nc"] = _build()
    return _CACHE["nc"]


def _run(u_nom: np.ndarray, obs: np.ndarray, trace: bool = False):
    from concourse.bass_utils import run_bass_kernel_spmd

    u_nom = np.asarray(u_nom, dtype=np.float32)
    obs = np.asarray(obs, dtype=np.float32)

    nc = _get_nc()
    in_maps = []
    for c in range(NCORES):
        s = slice(c * BC, (c + 1) * BC)
        uc = u_nom[s].reshape(P, NT, KC, 2)
        oc = obs[s].reshape(P, NT, KC, 6)
        # per-tile block layout: [gx gy | ux uy | vx vy]
        pk = np.stack(
            [oc[:, :, :, 2], oc[:, :, :, 3],
             uc[:, :, :, 0], uc[:, :, :, 1],
             oc[:, :, :, 4], oc[:, :, :, 5]],
            axis=2).reshape(P, NPER * 6)
        in_maps.append({"pk": np.ascontiguousarray(pk)})
    res = run_bass_kernel_spmd(nc, in_maps, core_ids=list(range(NCORES)),
                               trace=trace)
    out = np.empty((B, 2), dtype=np.float32)
    for c in range(NCORES):
        r = res.results[c]["out"].reshape(P, NT, 2, KC)
        out[c * BC:(c + 1) * BC] = np.transpose(r, (0, 1, 3, 2)).reshape(BC, 2)
    return out, res


def kernel(u_nom: np.ndarray, obs: np.ndarray) -> np.ndarray:
    return _run(u_nom, obs)[0]


if __name__ == "__main__":
    rng = np.random.default_rng(0)
    u = rng.standard_normal((B, 2), dtype=np.float32)
    o = rng.standard_normal((B, 6), dtype=np.float32)
    r = kernel(u, o)
    print(r.shape, r.dtype, r[:4])


# revision 31
# speedup vs baseline: 2.7926x; 1.4609x over previous
"""Trainium2 Bass kernel for nn_CBFLayer (batch CBF-QP safety filter).

Contract: kernel(u_nom, obs) takes FULL inputs (numpy), returns FULL output.
Internally: pure data-parallel shard of the batch across 8 NeuronCores.

Math (per sample, exact KKT of the QP  min |u-u_nom|^2 + LAM*s^2
s.t. a@u <= b+s, |u|^2 <= 1, s >= 0, with a = -2*g, g = p_rel):
  u = (u_nom + 2*t*g) * rho,  rho = min(1/||u_nom + 2*t*g||, 1)
with multiplier t per KKT case: t=0 (feasible), t2 (CBF active, ball
inactive; exact linear root), or the closed-form circle root z =
c*|w|/sqrt(1-c^2) pole-floored by (|C|/(2LAM))/S plus a deep-infeasible
branch t = 2*LAM*relu(-(b/2+sqrt(S))).  Seed-only accuracy ~7e-4 rel;
bf16 data path lands ~6e-3, inside the 2e-2 gate.

Implementation notes:
- inputs shipped bf16 in per-tile blocks [gx gy | gy -gx | ux uy | vx vy]
  so products fuse into wide DVE ops via broadcast/strided views;
- all transcendentals are Sqrt/Square/Relu/Abs/Copy activations (one
  table set: sqrt_and_others); divisions via reciprocal_approx_fast
  (f32-only custom DVE op - the only f32 islands in the pipeline);
- affine+mul chains fused with scalar_tensor_tensor (DVE-only opcode).
"""

import numpy as np
from ml_dtypes import bfloat16

B = 4194304
NCORES = 8
BC = B // NCORES            # 524288 samples per core
P = 128
NPER = BC // P              # 4096 samples per partition
KC = 1024                   # compute-tile samples per partition
NT = NPER // KC             # tiles per core

LAM = 10000.0
TOL = 1e-6

_CACHE = {}


def _build():
    import bass_rust as _bass_rust
    import concourse.bacc as bacc
    import concourse.mybir as mybir
    from concourse.tile import TileContext
    from concourse.hw_specs import get_activation_tables

    F32 = mybir.dt.float32
    BF16 = mybir.dt.bfloat16
    OP = mybir.AluOpType
    AF = mybir.ActivationFunctionType

    class _PinnedBacc(bacc.Bacc):
        """Activation-table chooser only sees sqrt_and_others (list order
        preserved so act_func_set_id indices stay aligned)."""

        def insert_act_table_loads(self):
            has_activation = any(
                isinstance(i, mybir.InstActivation)
                for b in self.main_func.blocks
                for i in b.instructions
            )
            if not has_activation:
                return
            tables = [
                (k, v if k == "sqrt_and_others" else set())
                for k, v in get_activation_tables(self.m.arch).items()
            ]
            _bass_rust.insert_act_table_loads(self, tables)

    nc = _PinnedBacc("TRN2", target_bir_lowering=False, debug=False)
    pk_in = nc.dram_tensor("pk", [P, NPER * 8], BF16, kind="ExternalInput").ap()
    out_d = nc.dram_tensor("out", [P, NPER * 2], BF16, kind="ExternalOutput").ap()

    def register_const(value):
        t = nc.alloc_sbuf_tensor(f"const-f32-{value}", [P, 1], F32)
        nc.gpsimd.memset(t.ap(), value)
        nc.const_aps.aps[(F32, value)] = t.ap()

    register_const(0.0)
    register_const(-1.0)
    register_const(1.0)
    register_const(-0.5 * TOL)
    nc.all_engine_barrier()

    with TileContext(nc) as tc:
        with (
            tc.tile_pool(name="io", bufs=2) as io,
            tc.tile_pool(name="wk", bufs=2) as wk,       # cross-stage values
            tc.tile_pool(name="ck", bufs=1) as ck,       # short-lived scratch
        ):
            def eng(e):
                return {"V": nc.vector, "G": nc.gpsimd}[e]

            def tt(e, out, a, b, op):
                eng(e).tensor_tensor(out, a, b, op)

            def ts(e, out, a, s1, op0, s2=None, op1=None):
                if op1 is None:
                    eng(e).tensor_scalar(out, a, s1, None, op0)
                else:
                    eng(e).tensor_scalar(out, a, s1, s2, op0, op1)

            def stt(out, in0, s, in1, op0, op1):
                nc.vector.scalar_tensor_tensor(out, in0, s, in1, op0, op1)

            def act(out, a, func, scale=1.0, bias=0.0):
                nc.scalar.activation(out, a, func, bias=bias, scale=scale)

            def rcp(out, in_):
                nc.vector.reciprocal_approx_fast(out=out, in_=in_)

            def bcast(ap, n):
                return ap.rearrange("p (o b) -> p o b", o=1).broadcast_to([P, 2, n])

            def T(name, n, dt):
                return ck.tile([P, n], dt, tag=name, name=name)

            def TW(name, n, dt):
                return wk.tile([P, n], dt, tag=name, name=name)

            def stage_a(i):
                # blocks [Gx Gy | Gy -Gx | ux uy | vx/2 vy/2], G = 2*p_rel
                st = {}
                pk_t = io.tile([P, 8 * KC], BF16, tag="pk_t")
                o_t = io.tile([P, 2 * KC], BF16, tag="o_t")
                nc.sync.dma_start(out=pk_t[:], in_=pk_in[:, i * 8 * KC:(i + 1) * 8 * KC])
                st["pk_t"], st["o_t"] = pk_t, o_t
                gsb = pk_t[:, 0:4 * KC]
                gb = pk_t[:, 0:2 * KC]
                ub = pk_t[:, 4 * KC:6 * KC]
                vb = pk_t[:, 6 * KC:8 * KC]
                st["gb"], st["ub"] = gb, ub
                # products: S'=|G|^2=4S, P'=G.u=2P, C'=GxU=2C, Vd=g.v, N=|u|^2
                gucr4 = T("gucr4", 4 * KC, BF16)
                tt("V", gucr4[:].rearrange("p (a b) -> p a b", a=2),
                   gsb.rearrange("p (a b) -> p a b", a=2), bcast(ub, 2 * KC), OP.mult)
                sq4 = T("sq4", 4 * KC, BF16)
                act(sq4[:].rearrange("p (a b) -> p a b", a=2),
                    pk_t[:].rearrange("p (a b) -> p a b", a=4)[:, 0::2, :], AF.Square)
                SN = TW("SN", 2 * KC, BF16)
                PC = TW("PC", 2 * KC, BF16)
                st["SN"], st["PC"] = SN, PC
                tt("V", SN[:, 0:KC], sq4[:, 0:KC], sq4[:, KC:2 * KC], OP.add)
                tt("V", SN[:, KC:2 * KC], sq4[:, 2 * KC:3 * KC], sq4[:, 3 * KC:4 * KC], OP.add)
                tt("V", PC[:, 0:KC], gucr4[:, 0:KC], gucr4[:, KC:2 * KC], OP.add)
                tt("V", PC[:, KC:2 * KC], gucr4[:, 2 * KC:3 * KC], gucr4[:, 3 * KC:4 * KC], OP.add)
                gvb = T("gvb", 2 * KC, BF16)
                tt("V", gvb[:], gb, vb, OP.mult)
                Vd = TW("Vd", KC, BF16)
                tt("G", Vd[:], gvb[:, 0:KC], gvb[:, KC:2 * KC], OP.add)
                st["Vd"] = Vd
                return st

            def solve(i, st):
                SN, PC, Vd = st["SN"], st["PC"], st["Vd"]
                gb, ub, o_t = st["gb"], st["ub"], st["o_t"]
                S_ = SN[:, 0:KC]
                N_ = SN[:, KC:2 * KC]
                P_ = PC[:, 0:KC]
                C_ = PC[:, KC:2 * KC]
                # -------- scalars --------
                S4m = T("S4m", KC, BF16); act(S4m[:], S_, AF.Copy, scale=0.25, bias=-1.0)
                b1 = T("b1", KC, BF16); tt("V", b1[:], S4m[:], Vd[:], OP.subtract)  # b/2
                b2t = T("b2t", KC, BF16); act(b2t[:], b1[:], AF.Copy, scale=2.0)    # b
                S32 = T("S32", KC, F32); act(S32[:], S_, AF.Copy)
                rS = T("rS", KC, F32); rcp(rS[:], S32[:])                  # 1/S'
                rSb = T("rSb", KC, BF16); act(rSb[:], rS[:], AF.Copy, scale=1.0 / LAM)
                rSbn = T("rSbn", KC, BF16); act(rSbn[:], rS[:], AF.Copy, scale=-1.0)
                sqS = T("sqS", KC, BF16); act(sqS[:], S_, AF.Sqrt, scale=0.25)
                isq = T("isq", KC, BF16); act(isq[:], rS[:], AF.Sqrt, scale=4.0)
                acr = T("acr", KC, BF16); act(acr[:], C_, AF.Abs)
                # -------- feas1: infeasible <=> P' + b*sqrt(max(N,1)) < 0 ------
                Nc = T("Nc", KC, BF16); act(Nc[:], N_, AF.Relu, bias=-1.0)
                sqNc = T("sqNc", KC, BF16); act(sqNc[:], Nc[:], AF.Sqrt, bias=1.0)
                rhs = T("rhs", KC, BF16); tt("G", rhs[:], b2t[:], sqNc[:], OP.mult)
                ff = T("ff", KC, BF16); tt("G", ff[:], P_, rhs[:], OP.add)
                nf1 = T("nf1", KC, BF16); ts("V", nf1[:], ff[:], -0.5 * TOL, OP.is_lt)
                # -------- case 2: t2 = -(P' + b)/(S' + 1e-4) --------
                den = T("den", KC, F32); act(den[:], S_, AF.Copy, bias=1e-4)
                rden = T("rden", KC, F32); rcp(rden[:], den[:])
                rdnb = T("rdnb", KC, BF16); act(rdnb[:], rden[:], AF.Copy, scale=-1.0)
                num = T("num", KC, BF16); tt("G", num[:], P_, b2t[:], OP.add)
                t2 = T("t2", KC, BF16); tt("V", t2[:], num[:], rdnb[:], OP.mult)
                w_ = T("w_", KC, BF16); tt("V", w_[:], t2[:], S_, OP.mult)
                P2t = T("P2t", KC, BF16); act(P2t[:], P_, AF.Copy, scale=2.0)
                w2 = T("w2", KC, BF16); tt("G", w2[:], P2t[:], w_[:], OP.add)
                x2 = T("x2", KC, BF16); tt("V", x2[:], t2[:], w2[:], OP.mult)
                n2 = T("n2", KC, BF16); tt("G", n2[:], x2[:], N_, OP.add)
                tq = T("w_", KC, BF16); act(tq[:], t2[:], AF.Copy, scale=-1e12)
                q1 = T("num", KC, BF16); tt("V", q1[:], tq[:], n2[:], OP.max)
                ok2 = T("ok2", KC, BF16); ts("V", ok2[:], q1[:], 1.0 + TOL, OP.is_le)
                # -------- case-3 closed-form seed --------
                beta = T("beta", KC, BF16); tt("V", beta[:], b1[:], isq[:], OP.mult)
                bsq = T("bsq", KC, BF16); act(bsq[:], beta[:], AF.Square)
                w2m = T("w2m", KC, BF16); act(w2m[:], bsq[:], AF.Copy, scale=-1.0, bias=1.0)
                ws2 = T("ws2", KC, BF16); tt("V", ws2[:], acr[:], rSb[:], OP.mult)
                w2c = T("w2c", KC, F32); stt(w2c[:], w2m[:], 1e-12, ws2[:], OP.max, OP.max)
                iw = T("iw", KC, F32); rcp(iw[:], w2c[:])
                rw = T("rw", KC, BF16); act(rw[:], iw[:], AF.Sqrt)
                km = T("km", KC, BF16); tt("V", km[:], acr[:], rw[:], OP.mult)
                km2 = T("km2", KC, BF16); tt("G", km2[:], km[:], beta[:], OP.mult)
                sm = T("sm", KC, BF16); tt("G", sm[:], P_, km2[:], OP.add)
                tmain = T("tmain", KC, BF16); tt("V", tmain[:], sm[:], rSbn[:], OP.mult)
                ta1 = T("ta1", KC, BF16); tt("G", ta1[:], b1[:], sqS[:], OP.add)
                talt = T("talt", KC, BF16); act(talt[:], ta1[:], AF.Relu, scale=-2.0 * LAM)
                t = TW("t", KC, BF16); tt("V", t[:], tmain[:], talt[:], OP.max)
                nc.vector.copy_predicated(t[:], ok2[:].bitcast(mybir.dt.uint16), t2[:])
                tt("G", t[:], t[:], nf1[:], OP.mult)
                st["t"] = t

            def solve2(i, st):
                gb, ub, o_t = st["gb"], st["ub"], st["o_t"]
                t, nf1 = st["t"], None
                axy = T("axy", 2 * KC, BF16)
                tt("V", axy[:].rearrange("p (o b) -> p o b", o=2),
                   bcast(t[:], KC), gb.rearrange("p (o b) -> p o b", o=2), OP.mult)
                sxy = TW("sxy", 2 * KC, BF16); tt("V", sxy[:], ub, axy[:], OP.add)
                sq2 = T("sq2", 2 * KC, BF16); act(sq2[:], sxy[:], AF.Square)
                nnf = T("nnf", KC, F32)
                stt(nnf[:], sq2[:, 0:KC], 1e-30, sq2[:, KC:2 * KC], OP.add, OP.add)
                inf = T("inf", KC, F32); rcp(inf[:], nnf[:])
                rho0 = T("rho0", KC, BF16); act(rho0[:], inf[:], AF.Sqrt)
                rho = TW("rho", KC, BF16); ts("V", rho[:], rho0[:], 1.0, OP.min)
                tt("V", o_t[:].rearrange("p (o b) -> p o b", o=2),
                   sxy[:].rearrange("p (o b) -> p o b", o=2), bcast(rho[:], KC), OP.mult)
                nc.sync.dma_start(out=out_d[:, i * 2 * KC:(i + 1) * 2 * KC], in_=o_t[:])

            sts = {0: stage_a(0)}
            for i in range(NT):
                solve(i, sts[i])
                if i + 1 < NT:
                    sts[i + 1] = stage_a(i + 1)
                solve2(i, sts.pop(i))
    nc.compile()
    return nc


def _get_nc():
    if "nc" not in _CACHE:
        _CACHE["nc"] = _build()
    return _CACHE["nc"]


def _run(u_nom: np.ndarray, obs: np.ndarray, trace: bool = False):
    from concourse.bass_utils import run_bass_kernel_spmd

    u_nom = np.asarray(u_nom, dtype=np.float32)
    obs = np.asarray(obs, dtype=np.float32)

    nc = _get_nc()
    in_maps = []
    for c in range(NCORES):
        s = slice(c * BC, (c + 1) * BC)
        uc = u_nom[s].reshape(P, NT, KC, 2).astype(bfloat16)
        oc = obs[s].reshape(P, NT, KC, 6).astype(bfloat16)
        gx = (2.0 * oc[:, :, :, 2].astype(np.float32)).astype(bfloat16)
        gy = (2.0 * oc[:, :, :, 3].astype(np.float32)).astype(bfloat16)
        # blocks: [Gx Gy | Gy -Gx | ux uy | vx/2 vy/2], G = 2*p_rel
        pk = np.stack(
            [gx, gy, gy, -gx,
             uc[:, :, :, 0], uc[:, :, :, 1],
             (0.5 * oc[:, :, :, 4].astype(np.float32)).astype(bfloat16),
             (0.5 * oc[:, :, :, 5].astype(np.float32)).astype(bfloat16)],
            axis=2).reshape(P, NPER * 8)
        in_maps.append({"pk": np.ascontiguousarray(pk)})
    res = run_bass_kernel_spmd(nc, in_maps, core_ids=list(range(NCORES)),
                               trace=trace)
    out = np.empty((B, 2), dtype=np.float32)
    for c in range(NCORES):
        r = np.asarray(res.results[c]["out"]).view(bfloat16).astype(np.float32)
        r = r.reshape(P, NT, 2, KC)
        out[c * BC:(c + 1) * BC] = np.transpose(r, (0, 1, 3, 2)).reshape(BC, 2)
    return out, res


def kernel(u_nom: np.ndarray, obs: np.ndarray) -> np.ndarray:
    return _run(u_nom, obs)[0]


if __name__ == "__main__":
    rng = np.random.default_rng(0)
    u = rng.standard_normal((B, 2), dtype=np.float32)
    o = rng.standard_normal((B, 6), dtype=np.float32)
    r = kernel(u, o)
    print(r.shape, r.dtype, r[:4])


# revision 32
# speedup vs baseline: 2.8419x; 1.0177x over previous
"""Trainium2 Bass kernel for nn_CBFLayer (batch CBF-QP safety filter).

Contract: kernel(u_nom, obs) takes FULL inputs (numpy), returns FULL output.
Internally: pure data-parallel shard of the batch across 8 NeuronCores.

Math (per sample, exact KKT of the QP  min |u-u_nom|^2 + LAM*s^2
s.t. a@u <= b+s, |u|^2 <= 1, s >= 0, with a = -2*g, g = p_rel):
  u = (u_nom + 2*t*g) * rho,  rho = min(1/||u_nom + 2*t*g||, 1)
with multiplier t per KKT case: t=0 (feasible), t2 (CBF active, ball
inactive; exact linear root), or the closed-form circle root z =
c*|w|/sqrt(1-c^2) pole-floored by (|C|/(2LAM))/S plus a deep-infeasible
branch t = 2*LAM*relu(-(b/2+sqrt(S))).  Seed-only accuracy ~7e-4 rel;
bf16 data path lands ~6e-3, inside the 2e-2 gate.

Implementation notes:
- inputs shipped bf16 in per-tile blocks [gx gy | gy -gx | ux uy | vx vy]
  so products fuse into wide DVE ops via broadcast/strided views;
- all transcendentals are Sqrt/Square/Relu/Abs/Copy activations (one
  table set: sqrt_and_others); divisions via reciprocal_approx_fast
  (f32-only custom DVE op - the only f32 islands in the pipeline);
- affine+mul chains fused with scalar_tensor_tensor (DVE-only opcode).
"""

import numpy as np
from ml_dtypes import bfloat16

B = 4194304
NCORES = 8
BC = B // NCORES            # 524288 samples per core
P = 128
NPER = BC // P              # 4096 samples per partition
KC = 1024                   # compute-tile samples per partition
NT = NPER // KC             # tiles per core

LAM = 10000.0
TOL = 1e-6

_CACHE = {}


def _build():
    import bass_rust as _bass_rust
    import concourse.bacc as bacc
    import concourse.mybir as mybir
    from concourse.tile import TileContext
    from concourse.hw_specs import get_activation_tables

    F32 = mybir.dt.float32
    BF16 = mybir.dt.bfloat16
    OP = mybir.AluOpType
    AF = mybir.ActivationFunctionType

    class _PinnedBacc(bacc.Bacc):
        """Activation-table chooser only sees sqrt_and_others (list order
        preserved so act_func_set_id indices stay aligned)."""

        def insert_act_table_loads(self):
            has_activation = any(
                isinstance(i, mybir.InstActivation)
                for b in self.main_func.blocks
                for i in b.instructions
            )
            if not has_activation:
                return
            tables = [
                (k, v if k == "sqrt_and_others" else set())
                for k, v in get_activation_tables(self.m.arch).items()
            ]
            _bass_rust.insert_act_table_loads(self, tables)

    nc = _PinnedBacc("TRN2", target_bir_lowering=False, debug=False)
    pk_in = nc.dram_tensor("pk", [P, NPER * 8], BF16, kind="ExternalInput").ap()
    out_d = nc.dram_tensor("out", [P, NPER * 2], BF16, kind="ExternalOutput").ap()

    def register_const(value):
        t = nc.alloc_sbuf_tensor(f"const-f32-{value}", [P, 1], F32)
        nc.gpsimd.memset(t.ap(), value)
        nc.const_aps.aps[(F32, value)] = t.ap()

    register_const(0.0)
    register_const(-1.0)
    register_const(1.0)
    register_const(-0.5 * TOL)
    nc.all_engine_barrier()

    with TileContext(nc) as tc:
        with (
            tc.tile_pool(name="io", bufs=2) as io,
            tc.tile_pool(name="wk", bufs=2) as wk,       # cross-stage values
            tc.tile_pool(name="ck", bufs=1) as ck,       # short-lived scratch
        ):
            def eng(e):
                return {"V": nc.vector, "G": nc.gpsimd}[e]

            def tt(e, out, a, b, op):
                eng(e).tensor_tensor(out, a, b, op)

            def ts(e, out, a, s1, op0, s2=None, op1=None):
                if op1 is None:
                    eng(e).tensor_scalar(out, a, s1, None, op0)
                else:
                    eng(e).tensor_scalar(out, a, s1, s2, op0, op1)

            def stt(out, in0, s, in1, op0, op1):
                nc.vector.scalar_tensor_tensor(out, in0, s, in1, op0, op1)

            def act(out, a, func, scale=1.0, bias=0.0):
                nc.scalar.activation(out, a, func, bias=bias, scale=scale)

            def rcp(out, in_):
                nc.vector.reciprocal_approx_fast(out=out, in_=in_)

            def bcast(ap, n):
                return ap.rearrange("p (o b) -> p o b", o=1).broadcast_to([P, 2, n])

            def T(name, n, dt):
                return ck.tile([P, n], dt, tag=name, name=name)

            def TW(name, n, dt):
                return wk.tile([P, n], dt, tag=name, name=name)

            def stage_a(i):
                # blocks [Gx Gy | Gy -Gx | ux uy | vx/2 vy/2], G = 2*p_rel
                st = {}
                pk_t = io.tile([P, 8 * KC], BF16, tag="pk_t")
                o_t = io.tile([P, 2 * KC], BF16, tag="o_t")
                nc.sync.dma_start(out=pk_t[:], in_=pk_in[:, i * 8 * KC:(i + 1) * 8 * KC])
                st["pk_t"], st["o_t"] = pk_t, o_t
                gsb = pk_t[:, 0:4 * KC]
                gb = pk_t[:, 0:2 * KC]
                ub = pk_t[:, 4 * KC:6 * KC]
                vb = pk_t[:, 6 * KC:8 * KC]
                st["gb"], st["ub"] = gb, ub
                # products: S'=|G|^2=4S, P'=G.u=2P, C'=GxU=2C, Vd=g.v, N=|u|^2
                gucr4 = T("gucr4", 4 * KC, BF16)
                tt("V", gucr4[:].rearrange("p (a b) -> p a b", a=2),
                   gsb.rearrange("p (a b) -> p a b", a=2), bcast(ub, 2 * KC), OP.mult)
                sq4 = T("sq4", 4 * KC, BF16)
                act(sq4[:].rearrange("p (a b) -> p a b", a=2),
                    pk_t[:].rearrange("p (a b) -> p a b", a=4)[:, 0::2, :], AF.Square)
                SN = TW("SN", 2 * KC, BF16)
                PC = TW("PC", 2 * KC, BF16)
                st["SN"], st["PC"] = SN, PC
                tt("V", SN[:, 0:KC], sq4[:, 0:KC], sq4[:, KC:2 * KC], OP.add)
                tt("V", SN[:, KC:2 * KC], sq4[:, 2 * KC:3 * KC], sq4[:, 3 * KC:4 * KC], OP.add)
                tt("V", PC[:, 0:KC], gucr4[:, 0:KC], gucr4[:, KC:2 * KC], OP.add)
                tt("V", PC[:, KC:2 * KC], gucr4[:, 2 * KC:3 * KC], gucr4[:, 3 * KC:4 * KC], OP.add)
                gvb = T("gvb", 2 * KC, BF16)
                tt("V", gvb[:], gb, vb, OP.mult)
                Vd = TW("Vd", KC, BF16)
                tt("G", Vd[:], gvb[:, 0:KC], gvb[:, KC:2 * KC], OP.add)
                st["Vd"] = Vd
                return st

            def solve(i, st):
                SN, PC, Vd = st["SN"], st["PC"], st["Vd"]
                gb, ub, o_t = st["gb"], st["ub"], st["o_t"]
                S_ = SN[:, 0:KC]
                N_ = SN[:, KC:2 * KC]
                P_ = PC[:, 0:KC]
                C_ = PC[:, KC:2 * KC]
                # ---- emission order tuned from gap profile: chain-critical
                # producers first per engine, off-chain masks late ----
                S2x = T("S2x", 2 * KC, F32)
                act(S2x[:, 0:KC], S_, AF.Copy)
                act(S2x[:, KC:2 * KC], S_, AF.Copy, bias=1e-4)
                S4m = T("S4m", KC, BF16); act(S4m[:], S_, AF.Copy, scale=0.25, bias=-1.0)
                b1 = T("b1", KC, BF16); tt("V", b1[:], S4m[:], Vd[:], OP.subtract)  # b/2
                rcp2 = T("rcp2", 2 * KC, F32); rcp(rcp2[:], S2x[:])
                rS = rcp2[:, 0:KC]
                rden = rcp2[:, KC:2 * KC]
                # S fillers while rcp runs, then post-rcp converts
                Nc = T("Nc", KC, BF16); act(Nc[:], N_, AF.Relu, bias=-1.0)
                sqNc = T("sqNc", KC, BF16); act(sqNc[:], Nc[:], AF.Sqrt, bias=1.0)
                b2t = T("b2t", KC, BF16); act(b2t[:], b1[:], AF.Copy, scale=2.0)    # b
                rSb = T("rSb", KC, BF16); act(rSb[:], rS, AF.Copy, scale=1.0 / LAM)
                rdnb = T("rdnb", KC, BF16); act(rdnb[:], rden, AF.Copy, scale=-1.0)
                isq = T("isq", KC, BF16); act(isq[:], rS, AF.Sqrt, scale=4.0)
                rSbn = T("rSbn", KC, BF16); act(rSbn[:], rS, AF.Copy, scale=-1.0)
                sqS = T("sqS", KC, BF16); act(sqS[:], S_, AF.Sqrt, scale=0.25)
                acr = T("acr", KC, BF16); act(acr[:], C_, AF.Abs)
                # G early: feas pieces + case-2 numerator
                rhs = T("rhs", KC, BF16); tt("G", rhs[:], b2t[:], sqNc[:], OP.mult)
                ff = T("ff", KC, BF16); tt("G", ff[:], P_, rhs[:], OP.add)
                num = T("num", KC, BF16); tt("G", num[:], P_, b2t[:], OP.add)
                # case-2 chain
                t2 = T("t2", KC, BF16); tt("V", t2[:], num[:], rdnb[:], OP.mult)
                w_ = T("w_", KC, BF16); tt("V", w_[:], t2[:], S_, OP.mult)
                P2t = T("P2t", KC, BF16); act(P2t[:], P_, AF.Copy, scale=2.0)
                w2 = T("w2", KC, BF16); tt("G", w2[:], P2t[:], w_[:], OP.add)
                x2 = T("x2", KC, BF16); tt("V", x2[:], t2[:], w2[:], OP.mult)
                n2 = T("n2", KC, BF16); tt("G", n2[:], x2[:], N_, OP.add)
                # seed chain
                beta = T("beta", KC, BF16); tt("V", beta[:], b1[:], isq[:], OP.mult)
                bsq = T("bsq", KC, BF16); act(bsq[:], beta[:], AF.Square)
                w2m = T("w2m", KC, BF16); act(w2m[:], bsq[:], AF.Copy, scale=-1.0, bias=1.0)
                ws2 = T("ws2", KC, BF16); tt("V", ws2[:], acr[:], rSb[:], OP.mult)
                w2c = T("w2c", KC, F32); stt(w2c[:], w2m[:], 1e-12, ws2[:], OP.max, OP.max)
                iw = T("iw", KC, F32); rcp(iw[:], w2c[:])
                rw = T("rw", KC, BF16); act(rw[:], iw[:], AF.Sqrt)
                km = T("km", KC, BF16); tt("V", km[:], acr[:], rw[:], OP.mult)
                ta1 = T("ta1", KC, BF16); tt("G", ta1[:], b1[:], sqS[:], OP.add)
                km2 = T("km2", KC, BF16); tt("G", km2[:], km[:], beta[:], OP.mult)
                sm = T("sm", KC, BF16); tt("G", sm[:], P_, km2[:], OP.add)
                # off-chain masks (late; consumers are cpred/gate)
                tq = T("w_", KC, BF16); act(tq[:], t2[:], AF.Copy, scale=-1e12)
                q1 = T("num", KC, BF16); tt("V", q1[:], tq[:], n2[:], OP.max)
                ok2 = T("ok2", KC, BF16); ts("V", ok2[:], q1[:], 1.0 + TOL, OP.is_le)
                nf1 = T("nf1", KC, BF16); ts("V", nf1[:], ff[:], -0.5 * TOL, OP.is_lt)
                # select
                tmain = T("tmain", KC, BF16); tt("V", tmain[:], sm[:], rSbn[:], OP.mult)
                talt = T("talt", KC, BF16); act(talt[:], ta1[:], AF.Relu, scale=-2.0 * LAM)
                t = T("t", KC, BF16); tt("V", t[:], tmain[:], talt[:], OP.max)
                nc.vector.copy_predicated(t[:], ok2[:].bitcast(mybir.dt.uint16), t2[:])
                tt("G", t[:], t[:], nf1[:], OP.mult)
                st["t"] = t

            def solve2(i, st):
                gb, ub, o_t = st["gb"], st["ub"], st["o_t"]
                t, nf1 = st["t"], None
                axy = T("axy", 2 * KC, BF16)
                tt("V", axy[:].rearrange("p (o b) -> p o b", o=2),
                   bcast(t[:], KC), gb.rearrange("p (o b) -> p o b", o=2), OP.mult)
                sxy = TW("sxy", 2 * KC, BF16); tt("V", sxy[:], ub, axy[:], OP.add)
                sq2 = T("sq2", 2 * KC, BF16); act(sq2[:], sxy[:], AF.Square)
                nnf = T("nnf", KC, F32)
                stt(nnf[:], sq2[:, 0:KC], 1e-30, sq2[:, KC:2 * KC], OP.add, OP.add)
                inf = T("inf", KC, F32); rcp(inf[:], nnf[:])
                rho0 = T("rho0", KC, BF16); act(rho0[:], inf[:], AF.Sqrt)
                rho = TW("rho", KC, BF16); ts("V", rho[:], rho0[:], 1.0, OP.min)
                tt("V", o_t[:].rearrange("p (o b) -> p o b", o=2),
                   sxy[:].rearrange("p (o b) -> p o b", o=2), bcast(rho[:], KC), OP.mult)
                nc.sync.dma_start(out=out_d[:, i * 2 * KC:(i + 1) * 2 * KC], in_=o_t[:])

            sts = {0: stage_a(0)}
            for i in range(NT):
                solve(i, sts[i])
                if i + 1 < NT:
                    sts[i + 1] = stage_a(i + 1)
                solve2(i, sts.pop(i))
    nc.compile()
    return nc


def _get_nc():
    if "nc" not in _CACHE:
        _CACHE["nc"] = _build()
    return _CACHE["nc"]


def _run(u_nom: np.ndarray, obs: np.ndarray, trace: bool = False):
    from concourse.bass_utils import run_bass_kernel_spmd

    u_nom = np.asarray(u_nom, dtype=np.float32)
    obs = np.asarray(obs, dtype=np.float32)

    nc = _get_nc()
    in_maps = []
    for c in range(NCORES):
        s = slice(c * BC, (c + 1) * BC)
        uc = u_nom[s].reshape(P, NT, KC, 2).astype(bfloat16)
        oc = obs[s].reshape(P, NT, KC, 6).astype(bfloat16)
        gx = (2.0 * oc[:, :, :, 2].astype(np.float32)).astype(bfloat16)
        gy = (2.0 * oc[:, :, :, 3].astype(np.float32)).astype(bfloat16)
        # blocks: [Gx Gy | Gy -Gx | ux uy | vx/2 vy/2], G = 2*p_rel
        pk = np.stack(
            [gx, gy, gy, -gx,
             uc[:, :, :, 0], uc[:, :, :, 1],
             (0.5 * oc[:, :, :, 4].astype(np.float32)).astype(bfloat16),
             (0.5 * oc[:, :, :, 5].astype(np.float32)).astype(bfloat16)],
            axis=2).reshape(P, NPER * 8)
        in_maps.append({"pk": np.ascontiguousarray(pk)})
    res = run_bass_kernel_spmd(nc, in_maps, core_ids=list(range(NCORES)),
                               trace=trace)
    out = np.empty((B, 2), dtype=np.float32)
    for c in range(NCORES):
        r = np.asarray(res.results[c]["out"]).view(bfloat16).astype(np.float32)
        r = r.reshape(P, NT, 2, KC)
        out[c * BC:(c + 1) * BC] = np.transpose(r, (0, 1, 3, 2)).reshape(BC, 2)
    return out, res


def kernel(u_nom: np.ndarray, obs: np.ndarray) -> np.ndarray:
    return _run(u_nom, obs)[0]


if __name__ == "__main__":
    rng = np.random.default_rng(0)
    u = rng.standard_normal((B, 2), dtype=np.float32)
    o = rng.standard_normal((B, 6), dtype=np.float32)
    r = kernel(u, o)
    print(r.shape, r.dtype, r[:4])


# revision 33
# speedup vs baseline: 3.0603x; 1.0768x over previous
"""Trainium2 Bass kernel for nn_CBFLayer (batch CBF-QP safety filter).

Contract: kernel(u_nom, obs) takes FULL inputs (numpy), returns FULL output.
Internally: pure data-parallel shard of the batch across 8 NeuronCores.

Math (per sample, exact KKT of the QP  min |u-u_nom|^2 + LAM*s^2
s.t. a@u <= b+s, |u|^2 <= 1, s >= 0, with a = -2*g, g = p_rel):
  u = (u_nom + 2*t*g) * rho,  rho = min(1/||u_nom + 2*t*g||, 1)
with multiplier t per KKT case: t=0 (feasible), t2 (CBF active, ball
inactive; exact linear root), or the closed-form circle root z =
c*|w|/sqrt(1-c^2) pole-floored by (|C|/(2LAM))/S plus a deep-infeasible
branch t = 2*LAM*relu(-(b/2+sqrt(S))).  Seed-only accuracy ~7e-4 rel;
bf16 data path lands ~6e-3, inside the 2e-2 gate.

Implementation notes:
- inputs shipped bf16 in per-tile blocks [gx gy | gy -gx | ux uy | vx vy]
  so products fuse into wide DVE ops via broadcast/strided views;
- all transcendentals are Sqrt/Square/Relu/Abs/Copy activations (one
  table set: sqrt_and_others); divisions via reciprocal_approx_fast
  (f32-only custom DVE op - the only f32 islands in the pipeline);
- affine+mul chains fused with scalar_tensor_tensor (DVE-only opcode).
"""

import numpy as np
from ml_dtypes import bfloat16

B = 4194304
NCORES = 8
BC = B // NCORES            # 524288 samples per core
P = 128
NPER = BC // P              # 4096 samples per partition
KC = 1024                   # compute-tile samples per partition
NT = NPER // KC             # tiles per core

LAM = 10000.0
TOL = 1e-6

_CACHE = {}


def _build():
    import bass_rust as _bass_rust
    import concourse.bacc as bacc
    import concourse.mybir as mybir
    from concourse.tile import TileContext
    from concourse.hw_specs import get_activation_tables

    F32 = mybir.dt.float32
    BF16 = mybir.dt.bfloat16
    OP = mybir.AluOpType
    AF = mybir.ActivationFunctionType

    class _PinnedBacc(bacc.Bacc):
        """Activation-table chooser only sees sqrt_and_others (list order
        preserved so act_func_set_id indices stay aligned)."""

        def insert_act_table_loads(self):
            has_activation = any(
                isinstance(i, mybir.InstActivation)
                for b in self.main_func.blocks
                for i in b.instructions
            )
            if not has_activation:
                return
            tables = [
                (k, v if k == "sqrt_and_others" else set())
                for k, v in get_activation_tables(self.m.arch).items()
            ]
            _bass_rust.insert_act_table_loads(self, tables)

    nc = _PinnedBacc("TRN2", target_bir_lowering=False, debug=False)
    pk_in = nc.dram_tensor("pk", [P, NPER * 8], BF16, kind="ExternalInput").ap()
    out_d = nc.dram_tensor("out", [P, NPER * 2], BF16, kind="ExternalOutput").ap()

    def register_const(value):
        t = nc.alloc_sbuf_tensor(f"const-f32-{value}", [P, 1], F32)
        nc.gpsimd.memset(t.ap(), value)
        nc.const_aps.aps[(F32, value)] = t.ap()

    register_const(0.0)
    register_const(-1.0)
    register_const(1.0)
    register_const(-0.5 * TOL)
    nc.all_engine_barrier()

    with TileContext(nc) as tc:
        with (
            tc.tile_pool(name="io", bufs=2) as io,
            tc.tile_pool(name="wk", bufs=2) as wk,       # cross-stage values
            tc.tile_pool(name="ck", bufs=1) as ck,       # short-lived scratch
        ):
            def eng(e):
                return {"V": nc.vector, "G": nc.gpsimd}[e]

            def tt(e, out, a, b, op):
                eng(e).tensor_tensor(out, a, b, op)

            def ts(e, out, a, s1, op0, s2=None, op1=None):
                if op1 is None:
                    eng(e).tensor_scalar(out, a, s1, None, op0)
                else:
                    eng(e).tensor_scalar(out, a, s1, s2, op0, op1)

            def stt(out, in0, s, in1, op0, op1):
                nc.vector.scalar_tensor_tensor(out, in0, s, in1, op0, op1)

            def act(out, a, func, scale=1.0, bias=0.0):
                nc.scalar.activation(out, a, func, bias=bias, scale=scale)

            def rcp(out, in_):
                nc.vector.reciprocal_approx_fast(out=out, in_=in_)

            def bcast(ap, n):
                return ap.rearrange("p (o b) -> p o b", o=1).broadcast_to([P, 2, n])

            def T(name, n, dt):
                return ck.tile([P, n], dt, tag=name, name=name)

            def TW(name, n, dt):
                return wk.tile([P, n], dt, tag=name, name=name)

            def stage_a(i):
                # blocks [Gx Gy | Gy -Gx | ux uy | vx/2 vy/2], G = 2*p_rel
                st = {}
                pk_t = io.tile([P, 8 * KC], BF16, tag="pk_t")
                o_t = io.tile([P, 2 * KC], BF16, tag="o_t")
                nc.sync.dma_start(out=pk_t[:], in_=pk_in[:, i * 8 * KC:(i + 1) * 8 * KC])
                st["pk_t"], st["o_t"] = pk_t, o_t
                gsb = pk_t[:, 0:4 * KC]
                gb = pk_t[:, 0:2 * KC]
                ub = pk_t[:, 4 * KC:6 * KC]
                vb = pk_t[:, 6 * KC:8 * KC]
                st["gb"], st["ub"] = gb, ub
                # products: S'=|G|^2=4S, P'=G.u=2P, C'=GxU=2C, Vd=g.v, N=|u|^2
                gucr4 = T("gucr4", 4 * KC, BF16)
                tt("V", gucr4[:].rearrange("p (a b) -> p a b", a=2),
                   gsb.rearrange("p (a b) -> p a b", a=2), bcast(ub, 2 * KC), OP.mult)
                sq4 = T("sq4", 4 * KC, BF16)
                act(sq4[:].rearrange("p (a b) -> p a b", a=2),
                    pk_t[:].rearrange("p (a b) -> p a b", a=4)[:, 0::2, :], AF.Square)
                SN = TW("SN", 2 * KC, BF16)
                PC = TW("PC", 2 * KC, BF16)
                st["SN"], st["PC"] = SN, PC
                tt("V", SN[:, 0:KC], sq4[:, 0:KC], sq4[:, KC:2 * KC], OP.add)
                tt("V", SN[:, KC:2 * KC], sq4[:, 2 * KC:3 * KC], sq4[:, 3 * KC:4 * KC], OP.add)
                tt("V", PC[:, 0:KC], gucr4[:, 0:KC], gucr4[:, KC:2 * KC], OP.add)
                tt("V", PC[:, KC:2 * KC], gucr4[:, 2 * KC:3 * KC], gucr4[:, 3 * KC:4 * KC], OP.add)
                gvb = T("gvb", 2 * KC, BF16)
                tt("V", gvb[:], gb, vb, OP.mult)
                Vd = TW("Vd", KC, BF16)
                tt("G", Vd[:], gvb[:, 0:KC], gvb[:, KC:2 * KC], OP.add)
                st["Vd"] = Vd
                return st

            def solve(i, st):
                SN, PC, Vd = st["SN"], st["PC"], st["Vd"]
                gb, ub, o_t = st["gb"], st["ub"], st["o_t"]
                S_ = SN[:, 0:KC]
                N_ = SN[:, KC:2 * KC]
                P_ = PC[:, 0:KC]
                C_ = PC[:, KC:2 * KC]
                # ---- emission order tuned from gap profile: chain-critical
                # producers first per engine, off-chain masks late ----
                S2x = T("S2x", 2 * KC, F32)
                act(S2x[:, 0:KC], S_, AF.Copy)
                act(S2x[:, KC:2 * KC], S_, AF.Copy, bias=1e-4)
                S4m = T("S4m", KC, BF16); act(S4m[:], S_, AF.Copy, scale=0.25, bias=-1.0)
                b1 = T("b1", KC, BF16); tt("V", b1[:], S4m[:], Vd[:], OP.subtract)  # b/2
                rcp2 = T("rcp2", 2 * KC, F32); rcp(rcp2[:], S2x[:])
                rS = rcp2[:, 0:KC]
                rden = rcp2[:, KC:2 * KC]
                # S fillers while rcp runs, then post-rcp converts
                Nc = T("Nc", KC, BF16); act(Nc[:], N_, AF.Relu, bias=-1.0)
                sqNc = T("sqNc", KC, BF16); act(sqNc[:], Nc[:], AF.Sqrt, bias=1.0)
                b2t = T("b2t", KC, BF16); act(b2t[:], b1[:], AF.Copy, scale=2.0)    # b
                rSb = T("rSb", KC, BF16); act(rSb[:], rS, AF.Copy, scale=1.0 / LAM)
                rdnb = T("rdnb", KC, BF16); act(rdnb[:], rden, AF.Copy, scale=-1.0)
                isq = T("isq", KC, BF16); act(isq[:], rS, AF.Sqrt, scale=4.0)
                rSbn = T("rSbn", KC, BF16); act(rSbn[:], rS, AF.Copy, scale=-1.0)
                sqS = T("sqS", KC, BF16); act(sqS[:], S_, AF.Sqrt, scale=0.25)
                acr = T("acr", KC, BF16); act(acr[:], C_, AF.Abs)
                # G early: feas pieces + case-2 numerator
                rhs = T("rhs", KC, BF16); tt("G", rhs[:], b2t[:], sqNc[:], OP.mult)
                ff = T("ff", KC, BF16); tt("G", ff[:], P_, rhs[:], OP.add)
                num = T("num", KC, BF16); tt("G", num[:], P_, b2t[:], OP.add)
                # case-2 chain
                t2 = T("t2", KC, BF16); tt("V", t2[:], num[:], rdnb[:], OP.mult)
                w_ = T("w_", KC, BF16); tt("V", w_[:], t2[:], S_, OP.mult)
                P2t = T("P2t", KC, BF16); act(P2t[:], P_, AF.Copy, scale=2.0)
                w2 = T("w2", KC, BF16); tt("G", w2[:], P2t[:], w_[:], OP.add)
                x2 = T("x2", KC, BF16); tt("V", x2[:], t2[:], w2[:], OP.mult)
                n2 = T("n2", KC, BF16); tt("G", n2[:], x2[:], N_, OP.add)
                # seed chain
                beta = T("beta", KC, BF16); tt("V", beta[:], b1[:], isq[:], OP.mult)
                bsq = T("bsq", KC, BF16); act(bsq[:], beta[:], AF.Square)
                w2m = T("w2m", KC, BF16); act(w2m[:], bsq[:], AF.Copy, scale=-1.0, bias=1.0)
                ws2 = T("ws2", KC, BF16); tt("V", ws2[:], acr[:], rSb[:], OP.mult)
                w2c = T("w2c", KC, F32); stt(w2c[:], w2m[:], 1e-12, ws2[:], OP.max, OP.max)
                iw = T("iw", KC, F32); rcp(iw[:], w2c[:])
                rw = T("rw", KC, BF16); act(rw[:], iw[:], AF.Sqrt)
                km = T("km", KC, BF16); tt("V", km[:], acr[:], rw[:], OP.mult)
                ta1 = T("ta1", KC, BF16); tt("G", ta1[:], b1[:], sqS[:], OP.add)
                km2 = T("km2", KC, BF16); tt("V", km2[:], km[:], beta[:], OP.mult)
                sm = T("sm", KC, BF16); tt("V", sm[:], P_, km2[:], OP.add)
                # off-chain masks (late; consumers are cpred/gate)
                tq = T("w_", KC, BF16); act(tq[:], t2[:], AF.Copy, scale=-1e12)
                q1 = T("num", KC, BF16); tt("V", q1[:], tq[:], n2[:], OP.max)
                ok2 = T("ok2", KC, BF16); ts("V", ok2[:], q1[:], 1.0 + TOL, OP.is_le)
                nf1 = T("nf1", KC, BF16); ts("V", nf1[:], ff[:], -0.5 * TOL, OP.is_lt)
                # select
                tmain = T("tmain", KC, BF16); tt("V", tmain[:], sm[:], rSbn[:], OP.mult)
                talt = T("talt", KC, BF16); act(talt[:], ta1[:], AF.Relu, scale=-2.0 * LAM)
                t = T("t", KC, BF16); tt("V", t[:], tmain[:], talt[:], OP.max)
                nc.vector.copy_predicated(t[:], ok2[:].bitcast(mybir.dt.uint16), t2[:])
                tt("V", t[:], t[:], nf1[:], OP.mult)
                st["t"] = t

            def solve2(i, st):
                gb, ub, o_t = st["gb"], st["ub"], st["o_t"]
                t, nf1 = st["t"], None
                axy = T("axy", 2 * KC, BF16)
                tt("V", axy[:].rearrange("p (o b) -> p o b", o=2),
                   bcast(t[:], KC), gb.rearrange("p (o b) -> p o b", o=2), OP.mult)
                sxy = TW("sxy", 2 * KC, BF16); tt("V", sxy[:], ub, axy[:], OP.add)
                sq2 = T("sq2", 2 * KC, BF16); tt("V", sq2[:], sxy[:], sxy[:], OP.mult)
                nnf = T("nnf", KC, F32)
                stt(nnf[:], sq2[:, 0:KC], 1e-30, sq2[:, KC:2 * KC], OP.add, OP.add)
                inf = T("inf", KC, F32); rcp(inf[:], nnf[:])
                rho0 = T("rho0", KC, BF16); act(rho0[:], inf[:], AF.Sqrt)
                rho = TW("rho", KC, BF16); ts("V", rho[:], rho0[:], 1.0, OP.min)
                tt("V", o_t[:].rearrange("p (o b) -> p o b", o=2),
                   sxy[:].rearrange("p (o b) -> p o b", o=2), bcast(rho[:], KC), OP.mult)
                nc.sync.dma_start(out=out_d[:, i * 2 * KC:(i + 1) * 2 * KC], in_=o_t[:])

            sts = {0: stage_a(0)}
            for i in range(NT):
                solve(i, sts[i])
                if i + 1 < NT:
                    sts[i + 1] = stage_a(i + 1)
                solve2(i, sts.pop(i))
    nc.compile()
    return nc


def _get_nc():
    if "nc" not in _CACHE:
        _CACHE["nc"] = _build()
    return _CACHE["nc"]


def _run(u_nom: np.ndarray, obs: np.ndarray, trace: bool = False):
    from concourse.bass_utils import run_bass_kernel_spmd

    u_nom = np.asarray(u_nom, dtype=np.float32)
    obs = np.asarray(obs, dtype=np.float32)

    nc = _get_nc()
    in_maps = []
    for c in range(NCORES):
        s = slice(c * BC, (c + 1) * BC)
        uc = u_nom[s].reshape(P, NT, KC, 2).astype(bfloat16)
        oc = obs[s].reshape(P, NT, KC, 6).astype(bfloat16)
        gx = (2.0 * oc[:, :, :, 2].astype(np.float32)).astype(bfloat16)
        gy = (2.0 * oc[:, :, :, 3].astype(np.float32)).astype(bfloat16)
        # blocks: [Gx Gy | Gy -Gx | ux uy | vx/2 vy/2], G = 2*p_rel
        pk = np.stack(
            [gx, gy, gy, -gx,
             uc[:, :, :, 0], uc[:, :, :, 1],
             (0.5 * oc[:, :, :, 4].astype(np.float32)).astype(bfloat16),
             (0.5 * oc[:, :, :, 5].astype(np.float32)).astype(bfloat16)],
            axis=2).reshape(P, NPER * 8)
        in_maps.append({"pk": np.ascontiguousarray(pk)})
    res = run_bass_kernel_spmd(nc, in_maps, core_ids=list(range(NCORES)),
                               trace=trace)
    out = np.empty((B, 2), dtype=np.float32)
    for c in range(NCORES):
        r = np.asarray(res.results[c]["out"]).view(bfloat16).astype(np.float32)
        r = r.reshape(P, NT, 2, KC)
        out[c * BC:(c + 1) * BC] = np.transpose(r, (0, 1, 3, 2)).reshape(BC, 2)
    return out, res


def kernel(u_nom: np.ndarray, obs: np.ndarray) -> np.ndarray:
    return _run(u_nom, obs)[0]


if __name__ == "__main__":
    rng = np.random.default_rng(0)
    u = rng.standard_normal((B, 2), dtype=np.float32)
    o = rng.standard_normal((B, 6), dtype=np.float32)
    r = kernel(u, o)
    print(r.shape, r.dtype, r[:4])
